# revision 34
# baseline (speedup 1.0000x reference)
"""Trainium2 Bass kernel for PhysicsAwareEmbedding GNN message passing.

Model (reference):
    nf = x[0,:,3:]                                   # [N, 6]
    h = gelu(nf @ lift_w1 + lift_b1) @ lift_w2 + lift_b2
    for l in range(2):
        neighbor = h @ neigh_w[l] + neigh_b[l]
        messages = neighbor[col] * edge_values[:, None]
        aggr = scatter_add(messages, row, N)
        self_f = h @ self_w[l] + self_b[l]
        u = gelu(concat([self_f, aggr]) @ gate_w1[l] + gate_b1[l]) @ gate_w2[l] + gate_b2[l]
        h = h + u
    out = layernorm(h) * g + b

Distribution: nodes sharded over 8 cores (12500/core, padded to 12544).
Edges sharded by destination. Per layer, each core computes its neighbor
shard, transposes it to node-major, AllGathers it, then gathers message
rows with indirect DMA and scatter-adds via one-hot matmuls into PSUM.

On-chip layout is feature-major (h^T: [128 C x nodes]) so dense matmuls
stream 512-column chunks against stationary [128,128] weights.
"""
import sys

if "/opt/trn_rl_repo" not in sys.path:
    sys.path.insert(0, "/opt/trn_rl_repo")

import numpy as np

import concourse.bacc as bacc
import concourse.bass as bass
import concourse.mybir as mybir
import concourse.tile as tile
from concourse.masks import make_identity

# problem constants (hardcoded per contract)
N = 100000
E = 1600000
C = 128
F = 6
L = 2
EPS = 1e-5

NCORES = 8
P = 128
SHARD = 12500               # real nodes per core
SHARD_PAD = 12544           # = 98 * 128
NBLK = SHARD_PAD // P       # 98 dest blocks per core
NG = 8 * SHARD_PAD          # gather table rows (allgathered)
CHUNK = 512                 # free-dim chunk for dense matmuls
CH = 2 * SHARD_PAD          # source chunk rows (25088, int16-addressable)
NCH = NG // CH              # 4 source chunks

FP = mybir.dt.float32


def _pick_gt(tiles_per_group):
    for d in range(8, 0, -1):
        if tiles_per_group % d == 0:
            return d
    return 1


def _host_preprocess(x, edge_index, edge_values,
                     lift_w1, lift_b1, lift_w2, lift_b2,
                     self_w, self_b, neigh_w, neigh_b,
                     gate_w1, gate_b1, gate_w2, gate_b2,
                     norm_g, norm_b):
    """Shard nodes/edges, sort edges by destination, fold biases."""
    row = np.asarray(edge_index[0], dtype=np.int64)
    col = np.asarray(edge_index[1], dtype=np.int64)
    ev = np.asarray(edge_values, dtype=np.float32)

    # node features, transposed + padded per core: nfT [6, 12544]
    nf = np.asarray(x[0, :, 3:], dtype=np.float32)  # [N, 6]
    nfT_shards = []
    for c in range(NCORES):
        blk = np.zeros((SHARD_PAD, F), np.float32)
        blk[:SHARD] = nf[c * SHARD:(c + 1) * SHARD]
        nfT_shards.append(np.ascontiguousarray(blk.T))  # [6, 12544]

    # gather-table row id for a global node: rank-local padded indexing
    gather_row = (col // SHARD) * SHARD_PAD + (col % SHARD)

    # per-core edge lists: grouped by source chunk, sorted by destination
    dest_core = row // SHARD
    percore = []
    t4 = 0
    for c in range(NCORES):
        sel = np.nonzero(dest_core == c)[0]
        r = (row[sel] - c * SHARD).astype(np.int64)   # local dest 0..12499
        g = gather_row[sel]
        v = ev[sel]
        ch = g // CH                                   # source chunk 0..3
        loc = (g % CH).astype(np.int64)                # chunk-local row
        # order: (chunk, dest) lexicographic
        order = np.lexsort((r, ch))
        r, v, ch, loc = r[order], v[order], ch[order], loc[order]
        blk = r // P
        cnt = np.zeros((NCH, NBLK), np.int64)
        np.add.at(cnt, (ch, blk), 1)
        percore.append((r, v, ch, loc, cnt))
        t4 = max(t4, int(np.ceil(cnt.max() / P)))
    T = t4                                  # tiles per (block, chunk)
    TPG = NBLK * T                          # tiles per group
    GT = _pick_gt(TPG)                      # tiles per gather instruction
    IPG = TPG // GT                         # instructions per group

    edge_meta = []
    for c in range(NCORES):
        r, v, ch, loc, cnt = percore[c]
        # slot of each edge within its (chunk, block) list
        flat_cnt = cnt.reshape(-1)
        starts = np.concatenate([[0], np.cumsum(flat_cnt)])[:-1].reshape(NCH, NBLK)
        blk = r // P
        pos = np.arange(len(r)) - starts[ch, blk]      # 0..n_bg-1 within group
        tile_in_group = blk * T + pos // P             # tile index within chunk-group
        part = pos % P
        colg = ch * TPG + tile_in_group                # global tile column

        idxflat = np.zeros((NCH, TPG * P), np.int16)   # pad -> row 0 of chunk
        rowloc = np.full((P, NCH * TPG), -1.0, np.float32)
        evv = np.zeros((P, NCH * TPG), np.float32)
        idxflat[ch, tile_in_group * P + part] = loc.astype(np.int16)
        rowloc[part, colg] = (r - blk * P).astype(np.float32)
        evv[part, colg] = v

        # wrap indices: per instruction (GT*P idxs), 16-part wrap, replicate x8
        # index i of instr k at [16*rep + i%16, k*(GT*8) + i//16]
        wi = idxflat.reshape(NCH * IPG, GT * 8, 16)    # [instr, col, 16]
        wi = wi.transpose(2, 0, 1).reshape(16, NCH * IPG * GT * 8)
        colidx16 = np.ascontiguousarray(np.tile(wi, (8, 1)))  # [128, cols]
        edge_meta.append((colidx16, rowloc, evv))

    # folded biases
    lift_w1 = np.asarray(lift_w1, np.float32)
    lift_w2 = np.asarray(lift_w2, np.float32)
    lb1 = np.asarray(lift_b1, np.float32)
    lb2 = np.asarray(lift_b2, np.float32)
    self_w = np.asarray(self_w, np.float32)
    neigh_w = np.asarray(neigh_w, np.float32)
    gw1 = np.asarray(gate_w1, np.float32)      # [L, 2C, C]
    gw1a = np.ascontiguousarray(gw1[:, :C, :])  # self part
    gw1b = np.ascontiguousarray(gw1[:, C:, :])  # aggr part
    gw2 = np.asarray(gate_w2, np.float32)
    nb = np.asarray(neigh_b, np.float32)        # [L, C] -> bias of allgathered neighbor
    # pre-gelu bias of gate1: self_b @ gw1a + gate_b1
    b1f = np.stack([np.asarray(self_b, np.float32)[l] @ gw1a[l]
                    + np.asarray(gate_b1, np.float32)[l] for l in range(L)])
    b2f = np.asarray(gate_b2, np.float32)       # [L, C]

    iota = np.broadcast_to(np.arange(P, dtype=np.float32), (P, P)).copy()
    g_bc = np.broadcast_to(np.asarray(norm_g, np.float32), (P, C)).copy()
    b_bc = np.broadcast_to(np.asarray(norm_b, np.float32), (P, C)).copy()

    consts = dict(
        w1=lift_w1, w2=lift_w2, lb1=lb1[:, None], lb2=lb2[:, None],
        selfw=self_w, neighw=neigh_w, gw1a=gw1a, gw1b=gw1b, gw2=gw2,
        nb=nb.T.copy(),       # [C, L] for per-partition bias slices
        b1f=b1f.T.copy(), b2f=b2f.T.copy(),
        iota=iota, g_bc=g_bc, b_bc=b_bc,
    )
    return nfT_shards, edge_meta, consts, T


def _build_program(T, debug=False):
    """Build the per-core Bass program (identical across cores)."""
    TPG = NBLK * T
    GT = _pick_gt(TPG)
    IPG = TPG // GT
    nc = bacc.Bacc(None, num_devices=NCORES)
    dt = FP

    nfT = nc.dram_tensor("nfT", [F, SHARD_PAD], dt, kind="ExternalInput")
    colidx_d = nc.dram_tensor("colidx", [P, NCH * IPG * GT * 8], mybir.dt.int16,
                              kind="ExternalInput")
    rowloc_d = nc.dram_tensor("rowloc", [P, NCH * TPG], dt, kind="ExternalInput")
    evv_d = nc.dram_tensor("evv", [P, NCH * TPG], dt, kind="ExternalInput")
    w1_d = nc.dram_tensor("w1", [F, C], dt, kind="ExternalInput")
    w2_d = nc.dram_tensor("w2", [C, C], dt, kind="ExternalInput")
    lb1_d = nc.dram_tensor("lb1", [C, 1], dt, kind="ExternalInput")
    lb2_d = nc.dram_tensor("lb2", [C, 1], dt, kind="ExternalInput")
    selfw_d = nc.dram_tensor("selfw", [L, C, C], dt, kind="ExternalInput")
    neighw_d = nc.dram_tensor("neighw", [L, C, C], dt, kind="ExternalInput")
    gw1a_d = nc.dram_tensor("gw1a", [L, C, C], dt, kind="ExternalInput")
    gw1b_d = nc.dram_tensor("gw1b", [L, C, C], dt, kind="ExternalInput")
    gw2_d = nc.dram_tensor("gw2", [L, C, C], dt, kind="ExternalInput")
    nb_d = nc.dram_tensor("nb", [C, L], dt, kind="ExternalInput")
    b1f_d = nc.dram_tensor("b1f", [C, L], dt, kind="ExternalInput")
    b2f_d = nc.dram_tensor("b2f", [C, L], dt, kind="ExternalInput")
    iota_d = nc.dram_tensor("iota", [P, P], dt, kind="ExternalInput")
    gbc_d = nc.dram_tensor("g_bc", [P, C], dt, kind="ExternalInput")
    bbc_d = nc.dram_tensor("b_bc", [P, C], dt, kind="ExternalInput")
    out_d = nc.dram_tensor("out", [SHARD_PAD, C], dt, kind="ExternalOutput")
    if debug:
        dbg_hlift = nc.dram_tensor("dbg_hlift", [P, SHARD_PAD], dt, kind="ExternalOutput")
        dbg_ngb = nc.dram_tensor("dbg_ngb", [2 * P, C], dt, kind="ExternalOutput")
        dbg_ngbf = nc.dram_tensor("dbg_ngbf", [8 * P, C], dt, kind="ExternalOutput")
        dbg_gat = nc.dram_tensor("dbg_gat", [P, GT * P], dt, kind="ExternalOutput")
        dbg_aggr = nc.dram_tensor("dbg_aggr", [P, SHARD_PAD], dt, kind="ExternalOutput")
        dbg_h1 = nc.dram_tensor("dbg_h1", [P, SHARD_PAD], dt, kind="ExternalOutput")

    AF = mybir.ActivationFunctionType
    OP = mybir.AluOpType

    # 512-column chunks over the shard
    chunks = [(i, min(CHUNK, SHARD_PAD - i)) for i in range(0, SHARD_PAD, CHUNK)]

    with tile.TileContext(nc) as tc:
        with (
            tc.tile_pool(name="persist", bufs=1) as pers,
            tc.tile_pool(name="work", bufs=3) as work,
            tc.tile_pool(name="gath", bufs=3) as gpool,
            tc.tile_pool(name="idx", bufs=3) as ipool,
            tc.tile_pool(name="st", bufs=4) as stpool,
            tc.tile_pool(name="psum", bufs=3, space="PSUM") as psum,
            tc.tile_pool(name="psumA", bufs=2, space="PSUM") as psumA,
            tc.tile_pool(name="psumT", bufs=2, space="PSUM") as psumT,
            tc.tile_pool(name="dram", bufs=1, space="DRAM") as dram,
        ):
            # ---- persistent SBUF state ----
            hT = pers.tile([P, SHARD_PAD], dt, tag="hT")
            aggrT = pers.tile([P, SHARD_PAD], dt, tag="aggrT")
            rowloc = pers.tile([P, NCH * TPG], dt, tag="rowloc")
            evv = pers.tile([P, NCH * TPG], dt, tag="evv")
            w1 = pers.tile([F, C], dt, tag="w1")
            w2 = pers.tile([C, C], dt, tag="w2")
            lb1 = pers.tile([C, 1], dt, tag="lb1")
            lb2 = pers.tile([C, 1], dt, tag="lb2")
            selfw = pers.tile([C, L * C], dt, tag="selfw")
            neighw = pers.tile([C, L * C], dt, tag="neighw")
            gw1a = pers.tile([C, L * C], dt, tag="gw1a")
            gw1b = pers.tile([C, L * C], dt, tag="gw1b")
            gw2 = pers.tile([C, L * C], dt, tag="gw2")
            nb = pers.tile([C, L], dt, tag="nb")
            b1f = pers.tile([C, L], dt, tag="b1f")
            b2f = pers.tile([C, L], dt, tag="b2f")
            iota = pers.tile([P, P], dt, tag="iota")
            g_bc = pers.tile([P, C], dt, tag="g_bc")
            b_bc = pers.tile([P, C], dt, tag="b_bc")
            ident = pers.tile([P, P], dt, tag="ident")
            eps_t = pers.tile([P, 1], dt, tag="eps")

            make_identity(nc, ident[:])
            nc.vector.memset(eps_t[:], float(EPS))
            nc.sync.dma_start(out=rowloc[:], in_=rowloc_d[:])
            nc.sync.dma_start(out=evv[:], in_=evv_d[:])
            nc.sync.dma_start(out=w1[:], in_=w1_d[:])
            nc.sync.dma_start(out=w2[:], in_=w2_d[:])
            nc.sync.dma_start(out=lb1[:], in_=lb1_d[:])
            nc.sync.dma_start(out=lb2[:], in_=lb2_d[:])
            for dst, src in ((selfw, selfw_d), (neighw, neighw_d), (gw1a, gw1a_d),
                             (gw1b, gw1b_d), (gw2, gw2_d)):
                nc.sync.dma_start(
                    out=dst[:].rearrange("c (l k) -> c l k", l=L),
                    in_=src[:].rearrange("l c k -> c l k"),
                )
            nc.sync.dma_start(out=nb[:], in_=nb_d[:])
            nc.sync.dma_start(out=b1f[:], in_=b1f_d[:])
            nc.sync.dma_start(out=b2f[:], in_=b2f_d[:])
            nc.sync.dma_start(out=iota[:], in_=iota_d[:])
            nc.sync.dma_start(out=g_bc[:], in_=gbc_d[:])
            nc.sync.dma_start(out=b_bc[:], in_=bbc_d[:])

            # ---- lift MLP: hT = gelu(nfT.T @ w1 + lb1).T ... feature-major ----
            for (c0, cw) in chunks:
                nft = work.tile([F, CHUNK], dt, tag="nft")
                nc.sync.dma_start(out=nft[:, :cw], in_=nfT[:, c0:c0 + cw])
                ps = psum.tile([P, CHUNK], mybir.dt.float32, tag="ps")
                nc.tensor.matmul(ps[:, :cw], w1[:], nft[:, :cw],
                                 start=True, stop=True)
                mid = work.tile([P, CHUNK], dt, tag="mid")
                nc.scalar.activation(mid[:, :cw], ps[:, :cw], AF.Gelu, bias=lb1[:])
                ps2 = psum.tile([P, CHUNK], mybir.dt.float32, tag="ps")
                nc.tensor.matmul(ps2[:, :cw], w2[:], mid[:, :cw],
                                 start=True, stop=True)
                nc.vector.tensor_scalar(
                    out=hT[:, c0:c0 + cw], in0=ps2[:, :cw],
                    scalar1=lb2[:], scalar2=None, op0=OP.add)

            if debug:
                nc.sync.dma_start(out=dbg_hlift[:], in_=hT[:])

            for l in range(L):
                # allgather buffers (DRAM) — one pair per layer; a Shared
                # buffer may only be written by a single collective inst
                ngb_shard = dram.tile([SHARD_PAD, C], dt, tag=f"ngb_shard{l}")
                ngb_full = dram.tile([NG, C], dt, tag=f"ngb_full{l}",
                                     addr_space="Shared")
                # ---- neighbor shard + transpose to node-major + write DRAM ----
                for (c0, cw) in chunks:
                    ps = psum.tile([P, CHUNK], mybir.dt.float32, tag="ps")
                    nc.tensor.matmul(ps[:, :cw], neighw[:, l * C:(l + 1) * C],
                                     hT[:, c0:c0 + cw], start=True, stop=True)
                    ngbT = work.tile([P, CHUNK], dt, tag="ngbT")
                    nc.scalar.activation(ngbT[:, :cw], ps[:, :cw], AF.Identity,
                                         bias=nb[:, l:l + 1])
                    # transpose [C, cw] -> [cw, C] in 128-blocks
                    for t0 in range(0, cw, P):
                        pt = psumT.tile([P, P], mybir.dt.float32, tag="pt")
                        nc.tensor.transpose(out=pt[:], in_=ngbT[:, t0:t0 + P],
                                            identity=ident[:])
                        ntile = work.tile([P, P], dt, tag="ntile")
                        nc.scalar.copy(ntile[:], pt[:])
                        nc.sync.dma_start(
                            out=ngb_shard[c0 + t0:c0 + t0 + P, :], in_=ntile[:])

                # ---- allgather ----
                nc.gpsimd.collective_compute(
                    "AllGather", OP.bypass,
                    replica_groups=[list(range(NCORES))],
                    ins=[ngb_shard[:]], outs=[ngb_full[:]],
                )
                if debug and l == 0:
                    nc.sync.dma_start(out=dbg_ngb[:], in_=ngb_shard[:2 * P, :])
                    for cc in range(NCORES):
                        nc.sync.dma_start(
                            out=dbg_ngbf[cc * P:(cc + 1) * P, :],
                            in_=ngb_full[cc * SHARD_PAD:cc * SHARD_PAD + P, :])

                # ---- message gather + one-hot scatter-add ----
                # 4 source chunks; per chunk: IPG dma_gathers of GT tiles each;
                # one-hot matmuls accumulate per (block, chunk) into packed
                # 4-block psum spans, then copy/add into aggrT.
                for g in range(NCH):
                    ps_span = None
                    for k in range(IPG):
                        idxt = ipool.tile([P, GT * 8], mybir.dt.int16, tag="idxt")
                        icol = (g * IPG + k) * GT * 8
                        nc.sync.dma_start(
                            out=idxt[:], in_=colidx_d[:, icol:icol + GT * 8])
                        gat = gpool.tile([P, GT * P], dt, tag="gat")
                        nc.gpsimd.dma_gather(
                            out_ap=gat[:].rearrange("p (t e) -> p t e", e=P),
                            in_ap=ngb_full[g * CH:(g + 1) * CH, :],
                            idxs_ap=idxt[:],
                            num_idxs=GT * P,
                            num_idxs_reg=GT * P,
                            elem_size=P,
                        )
                        if debug and l == 0 and g == 0 and k == 0:
                            nc.sync.dma_start(out=dbg_gat[:], in_=gat[:])
                        for s in range(GT):
                            j = k * GT + s          # tile within group
                            b = j // T              # dest block
                            t = j % T               # tile within (block, chunk)
                            span = b // 4
                            spanw = min(4 * P, SHARD_PAD - span * 4 * P)
                            if t == 0 and b % 4 == 0:
                                ps_span = psumA.tile([P, 4 * P], mybir.dt.float32,
                                                     tag="pa")
                            st = stpool.tile([P, P], dt, tag="st")
                            jc = g * TPG + j
                            nc.vector.scalar_tensor_tensor(
                                out=st[:], in0=iota[:],
                                scalar=rowloc[:, jc:jc + 1],
                                in1=evv[:, jc:jc + 1].to_broadcast([P, P]),
                                op0=OP.is_equal, op1=OP.mult)
                            cb = (b % 4) * P
                            nc.tensor.matmul(
                                ps_span[:, cb:cb + P],
                                gat[:, s * P:(s + 1) * P], st[:],
                                start=(t == 0), stop=(t == T - 1))
                            if t == T - 1 and (b % 4 == 3 or b == NBLK - 1):
                                a0 = span * 4 * P
                                if g == 0:
                                    nc.scalar.copy(aggrT[:, a0:a0 + spanw],
                                                   ps_span[:, :spanw])
                                else:
                                    nc.vector.tensor_add(
                                        aggrT[:, a0:a0 + spanw],
                                        aggrT[:, a0:a0 + spanw],
                                        ps_span[:, :spanw])

                # ---- self path + gate MLP + residual ----
                for (c0, cw) in chunks:
                    ps = psum.tile([P, CHUNK], mybir.dt.float32, tag="ps")
                    nc.tensor.matmul(ps[:, :cw], selfw[:, l * C:(l + 1) * C],
                                     hT[:, c0:c0 + cw], start=True, stop=True)
                    sf = work.tile([P, CHUNK], dt, tag="sf")
                    nc.scalar.copy(sf[:, :cw], ps[:, :cw])
                    pg = psum.tile([P, CHUNK], mybir.dt.float32, tag="ps")
                    nc.tensor.matmul(pg[:, :cw], gw1a[:, l * C:(l + 1) * C],
                                     sf[:, :cw], start=True, stop=False)
                    nc.tensor.matmul(pg[:, :cw], gw1b[:, l * C:(l + 1) * C],
                                     aggrT[:, c0:c0 + cw], start=False, stop=True)
                    um = work.tile([P, CHUNK], dt, tag="um")
                    nc.scalar.activation(um[:, :cw], pg[:, :cw], AF.Gelu,
                                         bias=b1f[:, l:l + 1])
                    pu = psum.tile([P, CHUNK], mybir.dt.float32, tag="ps")
                    nc.tensor.matmul(pu[:, :cw], gw2[:, l * C:(l + 1) * C],
                                     um[:, :cw], start=True, stop=True)
                    # h += u + b2f
                    nc.vector.scalar_tensor_tensor(
                        out=hT[:, c0:c0 + cw], in0=pu[:, :cw],
                        scalar=b2f[:, l:l + 1], in1=hT[:, c0:c0 + cw],
                        op0=OP.add, op1=OP.add)
                if debug and l == 0:
                    nc.sync.dma_start(out=dbg_aggr[:], in_=aggrT[:])
                    nc.sync.dma_start(out=dbg_h1[:], in_=hT[:])

            # ---- epilogue: transpose + layernorm + write out ----
            inv_c = 1.0 / C
            for b in range(NBLK):
                pt = psumT.tile([P, P], mybir.dt.float32, tag="pt")
                nc.tensor.transpose(out=pt[:], in_=hT[:, b * P:(b + 1) * P],
                                    identity=ident[:])
                sums = work.tile([P, 1], dt, tag="sums")
                nc.vector.reduce_sum(sums[:], pt[:], axis=mybir.AxisListType.X)
                negmu = work.tile([P, 1], dt, tag="negmu")
                nc.vector.tensor_scalar(out=negmu[:], in0=sums[:],
                                        scalar1=-inv_c, scalar2=None,
                                        op0=OP.mult)
                # (h - mu)^2 with per-partition bias; accum gives sum over C
                sq = work.tile([P, P], dt, tag="sq")
                varsum = work.tile([P, 1], dt, tag="varsum")
                nc.scalar.activation(sq[:], pt[:], AF.Square, bias=negmu[:],
                                     accum_out=varsum[:])
                sd = work.tile([P, 1], dt, tag="sd")
                nc.scalar.activation(sd[:], varsum[:], AF.Sqrt, scale=inv_c,
                                     bias=eps_t[:])
                rstd = work.tile([P, 1], dt, tag="rstd")
                nc.vector.reciprocal(rstd[:], sd[:])
                cen = work.tile([P, P], dt, tag="cen")
                nc.vector.tensor_scalar(out=cen[:], in0=pt[:], scalar1=negmu[:],
                                        scalar2=rstd[:], op0=OP.add,
                                        op1=OP.mult)
                scl = work.tile([P, P], dt, tag="scl")
                nc.vector.tensor_tensor(out=scl[:], in0=cen[:], in1=g_bc[:],
                                        op=OP.mult)
                ot = work.tile([P, P], dt, tag="ot")
                nc.vector.tensor_tensor(out=ot[:], in0=scl[:], in1=b_bc[:],
                                        op=OP.add)
                nc.sync.dma_start(out=out_d[b * P:(b + 1) * P, :], in_=ot[:])

    nc.finalize()
    return nc


_CACHE = {}
LAST_EXEC_NS = None
LAST_PROFILE = None
LAST_RESULTS = None


def _get_program(T, debug=False):
    key = (T, debug)
    if key not in _CACHE:
        _CACHE[key] = _build_program(T, debug=debug)
    return _CACHE[key]


def kernel(**inputs):
    import os as _os

    debug = bool(_os.environ.get("BASSK_DEBUG"))
    nfT_shards, edge_meta, consts, T = _host_preprocess(**inputs)
    nc = _get_program(T, debug=debug)

    in_maps = []
    for c in range(NCORES):
        colidx, rowloc, evv = edge_meta[c]
        m = dict(
            nfT=nfT_shards[c], colidx=colidx, rowloc=rowloc, evv=evv,
            **consts,
        )
        in_maps.append(m)

    import os

    if os.environ.get("BASSK_SIM"):
        import concourse.bass_interp as bi
        from concourse.bass_interp import MultiCoreSim

        # sim has no Gelu: run Identity then apply exact gelu on the output
        if not getattr(bi.InstructionExecutor, "_gelu_patched", False):
            _orig_act = bi.InstructionExecutor.visit_InstActivation

            def _patched_act(self, instruction, *, reg_snapshot=None):
                f = instruction.func
                if f == mybir.ActivationFunctionType.Gelu:
                    instruction.func = mybir.ActivationFunctionType.Identity
                    try:
                        _orig_act(self, instruction, reg_snapshot=reg_snapshot)
                    finally:
                        instruction.func = f
                    out_ap = instruction.outs[0]
                    view = self.view_ap(out_ap, bi.Direction.WRITE, instruction,
                                        reg_snapshot=reg_snapshot)
                    from scipy.special import erf

                    v = view[:].astype(np.float64)
                    view[:] = (0.5 * v * (1.0 + erf(v / np.sqrt(2.0)))).astype(
                        np.float32)
                else:
                    _orig_act(self, instruction, reg_snapshot=reg_snapshot)

            bi.InstructionExecutor.visit_InstActivation = _patched_act
            bi.InstructionExecutor._gelu_patched = True

        sim = MultiCoreSim(nc, NCORES)
        for c in range(NCORES):
            for k, v in in_maps[c].items():
                sim.cores[c].tensor(k)[:] = v
        sim.simulate()
        results = [{"out": sim.cores[c].mem_tensor("out")} for c in range(NCORES)]
    else:
        from concourse.bass_utils import run_bass_kernel_spmd

        trace = bool(os.environ.get("BASSK_TRACE"))
        res = run_bass_kernel_spmd(nc, in_maps, list(range(NCORES)), trace=trace)
        if trace:
            global LAST_EXEC_NS, LAST_PROFILE
            LAST_EXEC_NS = res.exec_time_ns
            LAST_PROFILE = res
            print(f"HW exec time: {res.exec_time_ns} ns")
        results = res.results
    global LAST_RESULTS
    LAST_RESULTS = results
    out = np.concatenate([results[c]["out"][:SHARD] for c in range(NCORES)],
                         axis=0)
    return out[None].astype(np.float32)
